# revision 18
# baseline (speedup 1.0000x reference)
"""Trainium2 Bass kernel for the pointer-network GRU decoder (nn_Decoder).

Strategy (data-parallel over batch, 8 cores, Bs = 128/8 = 16 per core):
  * Precompute W1eT[k, (b,n)] = W1 @ enc^T on the PE once.
  * Per decode step:
      - GRU gates as natural-layout PE matmuls: the tiny transposed state
        (xT/hT chunks, 16 cols) is the stationary operand and the gate weights
        stream as the moving operand (PE is weight-load bound otherwise: fp32
        LDWEIGHTS costs 2 cycles/col and the PE clock stays at 1.2 GHz).
        Biases are added on DVE from host-replicated tiles; sigmoid via tanh
        identity keeps the ACT table set fixed to exp/tanh the whole run.
      - attention: S = W1eT + W2h (broadcast add split DVE/GPSIMD),
        tanh(S) on ACT, then u[b,:] accumulated into one (16,256) PSUM tile via
        per-batch rank-v matmuls.
      - masked softmax / argmax (max8+max_index), entropy accumulators;
        log() calls deferred to one batched epilogue (single table switch).
      - prev_embed gather via indirect DMA from DRAM, transposed on PE.
"""
import numpy as np

try:
    import concourse.bass as bass  # noqa
except ImportError:  # harness env should have it; fall back to repo path
    import sys
    for p in ("/opt/trn_rl_repo", "/root/.axon_site/_ro/trn_rl_repo"):
        if p not in sys.path:
            sys.path.append(p)
    import concourse.bass as bass  # noqa

import concourse.tile as tile
import concourse.mybir as mybir
from concourse import bacc
from concourse.bass_utils import run_bass_kernel_spmd

FP = mybir.dt.float32
BF = mybir.dt.bfloat16
I32 = mybir.dt.int32
U32 = mybir.dt.uint32
NEG = -1e9

N_CORES = 8
B, N, H = 128, 256, 512
BS = B // N_CORES          # 16 batch rows per core
HC = H // 128              # 4 h chunks
T_STEPS = 32

AF = mybir.ActivationFunctionType
OP = mybir.AluOpType
AX = mybir.AxisListType

# how many of the 8 b-slots per (hc,half) the DVE takes for the broadcast add
# (the rest go to GPSIMD)
DVE_B = 5


def build(T=T_STEPS):
    nc = bacc.Bacc("TRN2", target_bir_lowering=False, debug=False)

    # ---- DRAM I/O (per-core shard shapes) ----
    encT_hi_in = nc.dram_tensor("encT_hi", [128, HC * BS * N], BF, kind="ExternalInput")
    encT_lo_in = nc.dram_tensor("encT_lo", [128, HC * BS * N], BF, kind="ExternalInput")
    enc_flat = nc.dram_tensor("encflat", [BS * N, H], FP, kind="ExternalInput")
    w1t_hi_in = nc.dram_tensor("w1t_hi", [128, HC * H], BF, kind="ExternalInput")
    w1t_lo_in = nc.dram_tensor("w1t_lo", [128, HC * H], BF, kind="ExternalInput")
    gw_hi_in = nc.dram_tensor("gw_hi", [128, 8 * 3 * H], BF, kind="ExternalInput")
    gw_lo_in = nc.dram_tensor("gw_lo", [128, 8 * 3 * H], BF, kind="ExternalInput")
    w2t_hi_in = nc.dram_tensor("w2t_hi", [128, HC * H], BF, kind="ExternalInput")
    w2t_lo_in = nc.dram_tensor("w2t_lo", [128, HC * H], BF, kind="ExternalInput")
    ve_in = nc.dram_tensor("ve", [128, HC * BS * BS], FP, kind="ExternalInput")
    ones_in = nc.dram_tensor("ones16", [1, BS], BF, kind="ExternalInput")
    brow_hi_in = nc.dram_tensor("brow_hi", [1, 3 * H], BF, kind="ExternalInput")
    brow_lo_in = nc.dram_tensor("brow_lo", [1, 3 * H], BF, kind="ExternalInput")
    bhn_row_hi_in = nc.dram_tensor("bhnrow_hi", [1, H], BF, kind="ExternalInput")
    bhn_row_lo_in = nc.dram_tensor("bhnrow_lo", [1, H], BF, kind="ExternalInput")
    iota_in = nc.dram_tensor("iota", [BS, N], FP, kind="ExternalInput")
    bvec_in = nc.dram_tensor("bvec", [BS, 1], FP, kind="ExternalInput")
    id_in = nc.dram_tensor("ident16", [BS, BS], FP, kind="ExternalInput")

    idx_out = nc.dram_tensor("idx_out", [BS, T], I32, kind="ExternalOutput")
    logp_out = nc.dram_tensor("logp_out", [BS, T], FP, kind="ExternalOutput")
    ent_out = nc.dram_tensor("ent_out", [BS, T], FP, kind="ExternalOutput")

    with tile.TileContext(nc) as tc:
        with (
            tc.tile_pool(name="consts", bufs=1) as consts,
            tc.tile_pool(name="w1e", bufs=1) as w1e_pool,
            tc.tile_pool(name="state", bufs=1) as state,
        ):
            # ---- load constants ----
            gw_hi = consts.tile([128, 8 * 3 * H], BF, tag="gw_hi")
            nc.sync.dma_start(gw_hi[:], gw_hi_in[:, :])
            gw_lo = consts.tile([128, 8 * 3 * H], BF, tag="gw_lo")
            nc.sync.dma_start(gw_lo[:], gw_lo_in[:, :])
            w2t_hi = consts.tile([128, HC * H], BF, tag="w2t_hi")
            nc.sync.dma_start(w2t_hi[:], w2t_hi_in[:, :])
            w2t_lo = consts.tile([128, HC * H], BF, tag="w2t_lo")
            nc.sync.dma_start(w2t_lo[:], w2t_lo_in[:, :])
            ve = consts.tile([128, HC * BS * BS], FP, tag="ve")
            nc.sync.dma_start(ve[:], ve_in[:, :])
            ones16 = consts.tile([1, BS], BF, tag="ones16")
            nc.sync.dma_start(ones16[:], ones_in[:, :])
            brow_hi = consts.tile([1, 3 * H], BF, tag="brow_hi")
            nc.sync.dma_start(brow_hi[:], brow_hi_in[:, :])
            brow_lo = consts.tile([1, 3 * H], BF, tag="brow_lo")
            nc.sync.dma_start(brow_lo[:], brow_lo_in[:, :])
            bhn_row_hi = consts.tile([1, H], BF, tag="bhnr_hi")
            nc.sync.dma_start(bhn_row_hi[:], bhn_row_hi_in[:, :])
            bhn_row_lo = consts.tile([1, H], BF, tag="bhnr_lo")
            nc.sync.dma_start(bhn_row_lo[:], bhn_row_lo_in[:, :])
            iota = consts.tile([BS, N], FP, tag="iota")
            nc.sync.dma_start(iota[:], iota_in[:, :])
            bvec = consts.tile([BS, 1], FP, tag="bvec")
            nc.sync.dma_start(bvec[:], bvec_in[:, :])
            ident16 = consts.tile([BS, BS], FP, tag="ident16")
            nc.sync.dma_start(ident16[:], id_in[:, :])

            w1e = [w1e_pool.tile([128, BS * N], FP, tag=f"w1e{kc}", name=f"w1e{kc}")
                   for kc in range(HC)]

            # ---- precompute W1eT = W1 @ encT ----
            with (
                tc.tile_pool(name="pre", bufs=1) as pre,
                tc.tile_pool(name="ec", bufs=2) as ecp,
                tc.tile_pool(name="ps_pre", bufs=2, space="PSUM") as psp,
            ):
                w1t_hi = pre.tile([128, HC * H], BF, tag="w1t_hi")
                nc.sync.dma_start(w1t_hi[:], w1t_hi_in[:, :])
                w1t_lo = pre.tile([128, HC * H], BF, tag="w1t_lo")
                nc.sync.dma_start(w1t_lo[:], w1t_lo_in[:, :])
                for f in range(8):
                    ech, ecl = [], []
                    for hc in range(HC):
                        eh = ecp.tile([128, 512], BF, tag=f"ech{hc}", name=f"ech{hc}")
                        nc.sync.dma_start(
                            eh[:], encT_hi_in[:, hc * BS * N + f * 512:hc * BS * N + (f + 1) * 512])
                        ech.append(eh)
                        el = ecp.tile([128, 512], BF, tag=f"ecl{hc}", name=f"ecl{hc}")
                        nc.sync.dma_start(
                            el[:], encT_lo_in[:, hc * BS * N + f * 512:hc * BS * N + (f + 1) * 512])
                        ecl.append(el)
                    for kc in range(HC):
                        pp = psp.tile([128, 512], FP, tag="pp")
                        nmm = 3 * HC
                        i = 0
                        for hc in range(HC):
                            ks = slice(hc * H + kc * 128, hc * H + (kc + 1) * 128)
                            for wt, e in ((w1t_hi, ech[hc]), (w1t_hi, ecl[hc]),
                                          (w1t_lo, ech[hc])):
                                nc.tensor.matmul(
                                    pp[:], wt[:, ks], e[:],
                                    start=(i == 0), stop=(i == nmm - 1))
                                i += 1
                        eng = nc.vector if (kc % 2 == 0) else nc.scalar
                        if eng is nc.vector:
                            eng.tensor_copy(w1e[kc][:, f * 512:(f + 1) * 512], pp[:])
                        else:
                            eng.copy(w1e[kc][:, f * 512:(f + 1) * 512], pp[:])

            # ---- step-loop pools (opened after precompute pools closed) ----
            import contextlib
            _stack = contextlib.ExitStack()
            work = _stack.enter_context(tc.tile_pool(name="work", bufs=2))
            bigS = _stack.enter_context(tc.tile_pool(name="bigS", bufs=2))
            bigT = _stack.enter_context(tc.tile_pool(name="bigT", bufs=2))
            ps_g = _stack.enter_context(tc.tile_pool(name="ps_g", bufs=1, space="PSUM"))
            ps_u = _stack.enter_context(tc.tile_pool(name="ps_u", bufs=1, space="PSUM"))
            ps_s = _stack.enter_context(tc.tile_pool(name="ps_s", bufs=1, space="PSUM"))

            # ---- persistent state ----
            hT = state.tile([128, HC * BS], FP, tag="hT")      # [hp, (c,b)]
            xT = state.tile([128, HC * BS], FP, tag="xT")
            h_nat = state.tile([BS, H], FP, tag="h_nat")       # [b, h]
            maskneg = state.tile([BS, N], FP, tag="maskneg")
            scol = state.tile([BS, T], FP, tag="scol")
            eucol = state.tile([BS, T], FP, tag="eucol")
            mxcol = state.tile([BS, T], FP, tag="mxcol")
            idxcol = state.tile([BS, T], I32, tag="idxcol")
            nc.vector.memset(hT[:], 0.0)
            nc.vector.memset(xT[:], 0.0)
            nc.vector.memset(h_nat[:], 0.0)
            nc.vector.memset(maskneg[:], 0.0)

            def split_bf16(src_fp, pfx, t):
                hi = state.tile([128, HC * BS], BF, tag=f"{pfx}h{(t % 2) + 1}",
                                name=f"{pfx}hi{t % 2}")
                nc.vector.tensor_copy(hi[:], src_fp[:])
                lo = state.tile([128, HC * BS], BF, tag=f"{pfx}l{(t % 2) + 1}",
                                name=f"{pfx}lo{t % 2}")
                nc.vector.tensor_tensor(lo[:], src_fp[:], hi[:], op=OP.subtract)
                return hi, lo

            xb_hi = state.tile([128, HC * BS], BF, tag="xbh0")
            xb_lo = state.tile([128, HC * BS], BF, tag="xbl0")
            hb_hi = state.tile([128, HC * BS], BF, tag="hbh0")
            hb_lo = state.tile([128, HC * BS], BF, tag="hbl0")
            nc.vector.memset(xb_hi[:], 0.0)
            nc.vector.memset(xb_lo[:], 0.0)
            nc.vector.memset(hb_hi[:], 0.0)
            nc.vector.memset(hb_lo[:], 0.0)

            # natural-layout gate matmuls: out[b, j]; stationary = xT/hT chunk
            # (16 bf16 cols -> cheap LDW), moving = bf16 gate weight rows.
            # fp32 accuracy via 3-pass hi/lo compensation (hi*hi + hi*lo + lo*hi).
            def g_passes(s_hi, s_lo):
                return ((s_hi, gw_hi), (s_hi, gw_lo), (s_lo, gw_hi))

            def emit_gates_h(rz_ps, hn_ps, in_ps, s_hi, s_lo):
                for jb in range(2):      # rz j-chunks (one PSUM bank each)
                    i = 0
                    for c in range(HC):
                        for st, wt in g_passes(s_hi, s_lo):
                            nc.tensor.matmul(
                                rz_ps[:, jb * 512:(jb + 1) * 512],
                                st[:, c * BS:(c + 1) * BS],
                                wt[:, (4 + c) * 3 * H + jb * 512:(4 + c) * 3 * H + (jb + 1) * 512],
                                start=(i == 0), stop=False)
                            i += 1
                    for br in (brow_hi, brow_lo):
                        nc.tensor.matmul(
                            rz_ps[:, jb * 512:(jb + 1) * 512], ones16[:],
                            br[:, jb * 512:(jb + 1) * 512],
                            start=False, stop=False)
                for k, br in enumerate((brow_hi, brow_lo)):
                    nc.tensor.matmul(in_ps[:, :], ones16[:],
                                     br[:, 2 * 512:3 * 512],
                                     start=(k == 0), stop=False)
                i = 0
                for c in range(HC):
                    for st, wt in g_passes(s_hi, s_lo):
                        nc.tensor.matmul(
                            hn_ps[:, :],
                            st[:, c * BS:(c + 1) * BS],
                            wt[:, (4 + c) * 3 * H + 2 * 512:(4 + c) * 3 * H + 3 * 512],
                            start=(i == 0), stop=False)
                        i += 1
                for k, br in enumerate((bhn_row_hi, bhn_row_lo)):
                    nc.tensor.matmul(hn_ps[:, :], ones16[:], br[:, :],
                                     start=False, stop=(k == 1))

            def emit_gates_x(rz_ps, in_ps, s_hi, s_lo):
                for jb in range(2):
                    i = 0
                    for c in range(HC):
                        for st, wt in g_passes(s_hi, s_lo):
                            nc.tensor.matmul(
                                rz_ps[:, jb * 512:(jb + 1) * 512],
                                st[:, c * BS:(c + 1) * BS],
                                wt[:, c * 3 * H + jb * 512:c * 3 * H + (jb + 1) * 512],
                                start=False, stop=(i == 3 * HC - 1))
                            i += 1

                i = 0
                for c in range(HC):
                    for st, wt in g_passes(s_hi, s_lo):
                        nc.tensor.matmul(
                            in_ps[:, :],
                            st[:, c * BS:(c + 1) * BS],
                            wt[:, c * 3 * H + 2 * 512:c * 3 * H + 3 * 512],
                            start=False, stop=(i == 3 * HC - 1))
                        i += 1

            rz_ps = ps_g.tile([BS, 2 * H], FP, tag="rz")
            hn_ps = ps_g.tile([BS, H], FP, tag="hn")
            in_ps = ps_g.tile([BS, H], FP, tag="inn")
            emit_gates_h(rz_ps, hn_ps, in_ps, hb_hi, hb_lo)

            for t in range(T):
                emit_gates_x(rz_ps, in_ps, xb_hi, xb_lo)

                # ---- gate nonlinearities, natural layout, chunked by c so
                # the h' transposes can start before the whole chain is done
                trz = work.tile([BS, 2 * H], FP, tag="trz", bufs=1)
                rz = work.tile([BS, 2 * H], FP, tag="rz_sb", bufs=1)
                rhn = work.tile([BS, H], FP, tag="rhn", bufs=1)
                npre = work.tile([BS, H], FP, tag="npre", bufs=1)
                nn_t = work.tile([BS, H], FP, tag="nn", bufs=1)
                hmn = work.tile([BS, H], FP, tag="hmn", bufs=1)
                h_new = state.tile([BS, H], FP, tag=f"hn{(t % 2) + 1}",
                                   name=f"hnat{t % 2}")
                tr_ps = ps_s.tile([128, 2 * HC * BS], FP, tag="htp")
                ht_ps = tr_ps[:, 0:HC * BS]
                for c in range(HC):
                    sl = slice(c * 128, (c + 1) * 128)
                    slz = slice(H + c * 128, H + (c + 1) * 128)
                    nc.scalar.activation(trz[:, sl], rz_ps[:, sl], AF.Tanh, scale=0.5)
                    nc.scalar.activation(trz[:, slz], rz_ps[:, slz], AF.Tanh, scale=0.5)
                    nc.vector.tensor_scalar(rz[:, sl], trz[:, sl], 0.5, 0.5,
                                            OP.mult, OP.add)
                    nc.vector.tensor_scalar(rz[:, slz], trz[:, slz], 0.5, 0.5,
                                            OP.mult, OP.add)
                    nc.vector.tensor_tensor(rhn[:, sl], rz[:, sl], hn_ps[:, sl],
                                            op=OP.mult)
                    nc.vector.tensor_tensor(npre[:, sl], rhn[:, sl], in_ps[:, sl],
                                            op=OP.add)
                    nc.scalar.activation(nn_t[:, sl], npre[:, sl], AF.Tanh)
                    nc.vector.tensor_tensor(hmn[:, sl], h_nat[:, sl], nn_t[:, sl],
                                            op=OP.subtract)
                    nc.vector.tensor_tensor(hmn[:, sl], rz[:, slz], hmn[:, sl],
                                            op=OP.mult)
                    nc.vector.tensor_tensor(h_new[:, sl], nn_t[:, sl], hmn[:, sl],
                                            op=OP.add)
                    nc.tensor.matmul(
                        ht_ps[:, c * BS:(c + 1) * BS],
                        h_new[:, sl],
                        ident16[:], is_transpose=True,
                        start=(c == 0), stop=(c == HC - 1))
                h_nat = h_new
                hT_new = state.tile([128, HC * BS], FP, tag=f"hT{(t % 2) + 1}",
                                    name=f"hT{t % 2}")
                nc.vector.tensor_copy(hT_new[:], ht_ps[:])
                hT = hT_new
                hb_hi, hb_lo = split_bf16(hT, "hb", t)

                # ---- W2h: natural matmul then transpose to [h, b] ----
                w2n_ps = ps_s.tile([BS, H], FP, tag="w2")
                i = 0
                for c in range(HC):
                    for st, wt in ((hb_hi, w2t_hi), (hb_hi, w2t_lo), (hb_lo, w2t_hi)):
                        nc.tensor.matmul(
                            w2n_ps[:, :],
                            st[:, c * BS:(c + 1) * BS],
                            wt[:, c * H:(c + 1) * H],
                            start=(i == 0), stop=(i == 3 * HC - 1))
                        i += 1
                w2n = work.tile([BS, H], FP, tag="w2n", bufs=1)
                nc.vector.tensor_copy(w2n[:], w2n_ps[:])
                w2_ps = tr_ps[:, HC * BS:2 * HC * BS]
                for c in range(HC):
                    nc.tensor.matmul(
                        w2_ps[:, c * BS:(c + 1) * BS],
                        w2n[:, c * 128:(c + 1) * 128],
                        ident16[:], is_transpose=True,
                        start=(c == 0), stop=(c == HC - 1))
                w2h = work.tile([128, HC * BS], FP, tag="w2h")
                nc.vector.tensor_copy(w2h[:], w2_ps[:])

                # ---- attention ----
                u_ps = ps_u.tile([BS, N], FP, tag="u")
                first = True
                for hc in range(HC):
                    for half in range(2):
                        S = bigS.tile([128, 8 * N], FP, tag="S")
                        s3 = S[:].rearrange("p (b n) -> p b n", n=N)
                        w1s = w1e[hc][:, half * 8 * N:(half + 1) * 8 * N].rearrange(
                            "p (b n) -> p b n", n=N)
                        wb = w2h[:, hc * BS + half * 8: hc * BS + half * 8 + 8]
                        pstep = wb.ap[0][0]
                        cstep = wb.ap[1][0]
                        nc.vector.tensor_tensor(
                            s3[:, 0:DVE_B, :], w1s[:, 0:DVE_B, :],
                            bass.AP(wb.tensor, wb.offset,
                                    [[pstep, 128], [cstep, DVE_B], [0, N]]),
                            op=OP.add)
                        nc.gpsimd.tensor_tensor(
                            s3[:, DVE_B:8, :], w1s[:, DVE_B:8, :],
                            bass.AP(wb.tensor, wb.offset + DVE_B * cstep,
                                    [[pstep, 128], [cstep, 8 - DVE_B], [0, N]]),
                            op=OP.add)
                        Tt = bigT.tile([128, 8 * N], FP, tag="Tt")
                        nc.scalar.activation(Tt[:], S[:], AF.Tanh)
                        for bl in range(8):
                            b = half * 8 + bl
                            last = (hc == HC - 1) and (half == 1) and (bl == 7)
                            nc.tensor.matmul(
                                u_ps[:, :],
                                ve[:, (hc * BS + b) * BS:(hc * BS + b + 1) * BS],
                                Tt[:, bl * N:(bl + 1) * N],
                                start=first, stop=last)
                            first = False

                # ---- argmax + gather issue (critical path first) ----
                up = work.tile([BS, N], FP, tag="up", bufs=1)
                nc.vector.tensor_tensor(up[:], u_ps[:], maskneg[:], op=OP.add)
                m8 = work.tile([BS, 8], FP, tag="m8")
                nc.vector.max(out=m8[:], in_=up[:])
                i8 = work.tile([BS, 8], U32, tag="i8")
                nc.vector.max_index(out=i8[:], in_max=m8[:], in_values=up[:])
                idxf = work.tile([BS, 1], FP, tag="idxf")
                nc.vector.tensor_copy(idxf[:], i8[:, 0:1])
                if t < T - 1:
                    offsf = work.tile([BS, 1], FP, tag="offsf")
                    nc.vector.tensor_scalar(offsf[:], idxf[:], bvec[:, 0:1], None, OP.add)
                    offs = work.tile([BS, 1], I32, tag="offs")
                    nc.vector.tensor_copy(offs[:], offsf[:])
                    prev = work.tile([BS, H], FP, tag="prev", bufs=1)
                    nc.gpsimd.indirect_dma_start(
                        out=prev[:], out_offset=None,
                        in_=enc_flat[:, :],
                        in_offset=bass.IndirectOffsetOnAxis(ap=offs[:, :1], axis=0))

                # next step's h-only gate matmuls: fills the PE while the
                # softmax stats + gather DMA are in flight
                if t < T - 1:
                    rz_ps = ps_g.tile([BS, 2 * H], FP, tag="rz")
                    hn_ps = ps_g.tile([BS, H], FP, tag="hn")
                    in_ps = ps_g.tile([BS, H], FP, tag="inn")
                    emit_gates_h(rz_ps, hn_ps, in_ps, hb_hi, hb_lo)

                # ---- softmax stats / mask update ----
                negmx = work.tile([BS, 1], FP, tag="negmx")
                nc.vector.tensor_scalar_mul(negmx[:], m8[:, 0:1], -1.0)
                nc.vector.tensor_copy(mxcol[:, t:t + 1], m8[:, 0:1])
                nc.vector.tensor_copy(idxcol[:, t:t + 1], i8[:, 0:1])
                ee = work.tile([BS, N], FP, tag="ee", bufs=1)
                nc.scalar.activation(ee[:], up[:], AF.Exp, bias=negmx[:, 0:1],
                                     accum_out=scol[:, t:t + 1])
                escr = work.tile([BS, N], FP, tag="escr", bufs=1)
                nc.vector.tensor_tensor(escr[:], ee[:], up[:], op=OP.mult)
                nc.vector.reduce_sum(out=eucol[:, t:t + 1], in_=escr[:], axis=AX.X)
                ohn = work.tile([BS, N], FP, tag="ohn", bufs=1)
                nc.vector.tensor_tensor(ohn[:], iota[:],
                                        idxf[:].to_broadcast([BS, N]),
                                        op=OP.is_equal)
                nc.vector.tensor_scalar_mul(ohn[:], ohn[:], NEG)
                nc.vector.tensor_tensor(maskneg[:], maskneg[:], ohn[:], op=OP.add)

                # ---- transpose gathered prev_embed ----
                if t < T - 1:
                    xt_ps = ps_s.tile([128, HC * BS], FP, tag="xtp")
                    for c in range(HC):
                        nc.tensor.matmul(
                            xt_ps[:, c * BS:(c + 1) * BS],
                            prev[:, c * 128:(c + 1) * 128],
                            ident16[:], is_transpose=True,
                            start=(c == 0), stop=(c == HC - 1))
                    xT_new = state.tile([128, HC * BS], FP, tag=f"xT{(t % 2) + 1}",
                                        name=f"xT{t % 2}")
                    nc.vector.tensor_copy(xT_new[:], xt_ps[:])
                    xT = xT_new
                    xb_hi, xb_lo = split_bf16(xT, "xb", t)

            # ---- epilogue: logs, entropy, outputs ----
            logs = state.tile([BS, T], FP, tag="logs")
            nc.scalar.activation(logs[:], scol[:], AF.Ln)
            rs = state.tile([BS, T], FP, tag="rs")
            nc.vector.reciprocal(rs[:], scol[:])
            t1 = state.tile([BS, T], FP, tag="t1")
            nc.vector.tensor_tensor(t1[:], mxcol[:], scol[:], op=OP.mult)
            nc.vector.tensor_tensor(t1[:], eucol[:], t1[:], op=OP.subtract)
            nc.vector.tensor_tensor(t1[:], t1[:], rs[:], op=OP.mult)
            ent_sb = state.tile([BS, T], FP, tag="ent")
            nc.vector.tensor_tensor(ent_sb[:], logs[:], t1[:], op=OP.subtract)
            logp_sb = state.tile([BS, T], FP, tag="logp")
            nc.vector.tensor_scalar_mul(logp_sb[:], logs[:], -1.0)
            nc.sync.dma_start(idx_out[:, :], idxcol[:])
            nc.sync.dma_start(logp_out[:, :], logp_sb[:])
            nc.sync.dma_start(ent_out[:, :], ent_sb[:])
            _stack.close()

    return nc


def _split16(a):
    import ml_dtypes
    hi = a.astype(ml_dtypes.bfloat16)
    lo = (a.astype(np.float32) - hi.astype(np.float32)).astype(ml_dtypes.bfloat16)
    return hi, lo


def _host_prep(enc_shard, w_ih, w_hh, b_ih, b_hh, W1, W2, v):
    """Build the per-core input map (enc_shard: (BS, N, H))."""
    f32 = lambda x: np.ascontiguousarray(x, dtype=np.float32)
    encT = f32(enc_shard.transpose(2, 0, 1).reshape(HC, 128, BS * N)
               .transpose(1, 0, 2).reshape(128, HC * BS * N))
    encT_hi, encT_lo = _split16(encT)
    enc_flat = f32(enc_shard.reshape(BS * N, H))
    w1t = f32(W1.T.reshape(HC, 128, H).transpose(1, 0, 2).reshape(128, HC * H))
    w1t_hi, w1t_lo = _split16(w1t)
    gws = np.concatenate([w_ih.T, w_hh.T], axis=0)  # (1024, 1536)
    gw = f32(gws.reshape(8, 128, 3 * H).transpose(1, 0, 2).reshape(128, 8 * 3 * H))
    gw_hi, gw_lo = _split16(gw)
    w2t = f32(W2.T.reshape(HC, 128, H).transpose(1, 0, 2).reshape(128, HC * H))
    w2t_hi, w2t_lo = _split16(w2t)
    ve = np.zeros((128, HC, BS, BS), np.float32)
    for hc in range(HC):
        for b in range(BS):
            ve[:, hc, b, b] = v[hc * 128:(hc + 1) * 128]
    ve = f32(ve.reshape(128, HC * BS * BS))
    # bias rows: rz part uses b_ih+b_hh; n part (cols 1024:1536) uses b_ih only
    brow = f32(np.concatenate([(b_ih + b_hh)[:2 * H], b_ih[2 * H:]])[None, :])
    brow_hi, brow_lo = _split16(brow)
    bhnrow = f32(b_hh[2 * H:][None, :])
    bhnrow_hi, bhnrow_lo = _split16(bhnrow)
    import ml_dtypes
    ones16 = np.ones((1, BS), ml_dtypes.bfloat16)
    iota = f32(np.broadcast_to(np.arange(N, dtype=np.float32)[None, :], (BS, N)))
    bvec = f32((np.arange(BS, dtype=np.float32) * N)[:, None])
    ident16 = np.eye(BS, dtype=np.float32)
    return {
        "encT_hi": encT_hi, "encT_lo": encT_lo, "encflat": enc_flat,
        "w1t_hi": w1t_hi, "w1t_lo": w1t_lo,
        "gw_hi": gw_hi, "gw_lo": gw_lo, "w2t_hi": w2t_hi, "w2t_lo": w2t_lo,
        "ve": ve, "ones16": ones16, "brow_hi": brow_hi, "brow_lo": brow_lo,
        "bhnrow_hi": bhnrow_hi, "bhnrow_lo": bhnrow_lo,
        "iota": iota, "bvec": bvec, "ident16": ident16,
    }


_NC_CACHE = {}


def kernel(encoder_outputs, w_ih, w_hh, b_ih, b_hh, W1, W2, v, decode_steps):
    T = int(decode_steps)
    enc = np.asarray(encoder_outputs, dtype=np.float32)
    args = [np.asarray(a, dtype=np.float32)
            for a in (w_ih, w_hh, b_ih, b_hh, W1, W2, v)]
    assert enc.shape == (B, N, H)

    if T not in _NC_CACHE:
        nc = build(T)
        nc.finalize()
        _NC_CACHE[T] = nc
    nc = _NC_CACHE[T]

    in_maps = []
    for c in range(N_CORES):
        shard = enc[c * BS:(c + 1) * BS]
        in_maps.append(_host_prep(shard, *args))

    res = run_bass_kernel_spmd(nc, in_maps, core_ids=list(range(N_CORES)))
    outs = res.results
    idx = np.concatenate([o["idx_out"] for o in outs], axis=0).astype(np.int32)
    logp = np.concatenate([o["logp_out"] for o in outs], axis=0).astype(np.float32)
    ent = np.concatenate([o["ent_out"] for o in outs], axis=0).astype(np.float32)
    return idx, logp, ent


# revision 19
# speedup vs baseline: 1.1958x; 1.1958x over previous
"""Trainium2 Bass kernel for the pointer-network GRU decoder (nn_Decoder).

Strategy (data-parallel over batch, 8 cores, Bs = 128/8 = 16 per core):
  * Precompute W1eT[k, (b,n)] = W1 @ enc^T on the PE once.
  * Per decode step:
      - GRU gates as natural-layout PE matmuls: the tiny transposed state
        (xT/hT chunks, 16 cols) is the stationary operand and the gate weights
        stream as the moving operand (PE is weight-load bound otherwise: fp32
        LDWEIGHTS costs 2 cycles/col and the PE clock stays at 1.2 GHz).
        Biases are added on DVE from host-replicated tiles; sigmoid via tanh
        identity keeps the ACT table set fixed to exp/tanh the whole run.
      - attention: S = W1eT + W2h (broadcast add split DVE/GPSIMD),
        tanh(S) on ACT, then u[b,:] accumulated into one (16,256) PSUM tile via
        per-batch rank-v matmuls.
      - masked softmax / argmax (max8+max_index), entropy accumulators;
        log() calls deferred to one batched epilogue (single table switch).
      - prev_embed gather via indirect DMA from DRAM, transposed on PE.
"""
import numpy as np

try:
    import concourse.bass as bass  # noqa
except ImportError:  # harness env should have it; fall back to repo path
    import sys
    for p in ("/opt/trn_rl_repo", "/root/.axon_site/_ro/trn_rl_repo"):
        if p not in sys.path:
            sys.path.append(p)
    import concourse.bass as bass  # noqa

import concourse.tile as tile
import concourse.mybir as mybir
from concourse import bacc
from concourse.bass_utils import run_bass_kernel_spmd

FP = mybir.dt.float32
BF = mybir.dt.bfloat16
I32 = mybir.dt.int32
U32 = mybir.dt.uint32
NEG = -1e9

N_CORES = 8
B, N, H = 128, 256, 512
BS = B // N_CORES          # 16 batch rows per core
HC = H // 128              # 4 h chunks
T_STEPS = 32

AF = mybir.ActivationFunctionType
OP = mybir.AluOpType
AX = mybir.AxisListType

# how many of the 8 b-slots per (hc,half) the DVE takes for the broadcast add
# (the rest go to GPSIMD)
DVE_B = 5


def build(T=T_STEPS):
    nc = bacc.Bacc("TRN2", target_bir_lowering=False, debug=False)

    # ---- DRAM I/O (per-core shard shapes) ----
    encT_hi_in = nc.dram_tensor("encT_hi", [128, HC * BS * N], BF, kind="ExternalInput")
    encT_lo_in = nc.dram_tensor("encT_lo", [128, HC * BS * N], BF, kind="ExternalInput")
    enc_flat = nc.dram_tensor("encflat", [BS * N, H], FP, kind="ExternalInput")
    w1t_hi_in = nc.dram_tensor("w1t_hi", [128, HC * H], BF, kind="ExternalInput")
    w1t_lo_in = nc.dram_tensor("w1t_lo", [128, HC * H], BF, kind="ExternalInput")
    gw_hi_in = nc.dram_tensor("gw_hi", [128, 8 * 3 * H], BF, kind="ExternalInput")
    gw_lo_in = nc.dram_tensor("gw_lo", [128, 8 * 3 * H], BF, kind="ExternalInput")
    w2t_hi_in = nc.dram_tensor("w2t_hi", [128, HC * H], BF, kind="ExternalInput")
    w2t_lo_in = nc.dram_tensor("w2t_lo", [128, HC * H], BF, kind="ExternalInput")
    ve_in = nc.dram_tensor("ve", [128, HC * BS * BS], FP, kind="ExternalInput")
    ones_in = nc.dram_tensor("ones16", [1, BS], BF, kind="ExternalInput")
    brow_hi_in = nc.dram_tensor("brow_hi", [1, 3 * H], BF, kind="ExternalInput")
    brow_lo_in = nc.dram_tensor("brow_lo", [1, 3 * H], BF, kind="ExternalInput")
    bhn_row_hi_in = nc.dram_tensor("bhnrow_hi", [1, H], BF, kind="ExternalInput")
    bhn_row_lo_in = nc.dram_tensor("bhnrow_lo", [1, H], BF, kind="ExternalInput")
    iota_in = nc.dram_tensor("iota", [BS, N], FP, kind="ExternalInput")
    bvec_in = nc.dram_tensor("bvec", [BS, 1], FP, kind="ExternalInput")
    id_in = nc.dram_tensor("ident16", [BS, BS], FP, kind="ExternalInput")

    idx_out = nc.dram_tensor("idx_out", [BS, T], I32, kind="ExternalOutput")
    logp_out = nc.dram_tensor("logp_out", [BS, T], FP, kind="ExternalOutput")
    ent_out = nc.dram_tensor("ent_out", [BS, T], FP, kind="ExternalOutput")

    with tile.TileContext(nc) as tc:
        with (
            tc.tile_pool(name="consts", bufs=1) as consts,
            tc.tile_pool(name="w1e", bufs=1) as w1e_pool,
            tc.tile_pool(name="state", bufs=1) as state,
        ):
            # ---- load constants ----
            gw_hi = consts.tile([128, 8 * 3 * H], BF, tag="gw_hi")
            nc.sync.dma_start(gw_hi[:], gw_hi_in[:, :])
            gw_lo = consts.tile([128, 8 * 3 * H], BF, tag="gw_lo")
            nc.sync.dma_start(gw_lo[:], gw_lo_in[:, :])
            w2t_hi = consts.tile([128, HC * H], BF, tag="w2t_hi")
            nc.sync.dma_start(w2t_hi[:], w2t_hi_in[:, :])
            w2t_lo = consts.tile([128, HC * H], BF, tag="w2t_lo")
            nc.sync.dma_start(w2t_lo[:], w2t_lo_in[:, :])
            ve = consts.tile([128, HC * BS * BS], FP, tag="ve")
            nc.sync.dma_start(ve[:], ve_in[:, :])
            ones16 = consts.tile([1, BS], BF, tag="ones16")
            nc.sync.dma_start(ones16[:], ones_in[:, :])
            brow_hi = consts.tile([1, 3 * H], BF, tag="brow_hi")
            nc.sync.dma_start(brow_hi[:], brow_hi_in[:, :])
            brow_lo = consts.tile([1, 3 * H], BF, tag="brow_lo")
            nc.sync.dma_start(brow_lo[:], brow_lo_in[:, :])
            bhn_row_hi = consts.tile([1, H], BF, tag="bhnr_hi")
            nc.sync.dma_start(bhn_row_hi[:], bhn_row_hi_in[:, :])
            bhn_row_lo = consts.tile([1, H], BF, tag="bhnr_lo")
            nc.sync.dma_start(bhn_row_lo[:], bhn_row_lo_in[:, :])
            iota = consts.tile([BS, N], FP, tag="iota")
            nc.sync.dma_start(iota[:], iota_in[:, :])
            bvec = consts.tile([BS, 1], FP, tag="bvec")
            nc.sync.dma_start(bvec[:], bvec_in[:, :])
            ident16 = consts.tile([BS, BS], FP, tag="ident16")
            nc.sync.dma_start(ident16[:], id_in[:, :])

            w1e = [w1e_pool.tile([128, BS * N], FP, tag=f"w1e{kc}", name=f"w1e{kc}")
                   for kc in range(HC)]

            # ---- precompute W1eT = W1 @ encT ----
            with (
                tc.tile_pool(name="pre", bufs=1) as pre,
                tc.tile_pool(name="ec", bufs=2) as ecp,
                tc.tile_pool(name="ps_pre", bufs=2, space="PSUM") as psp,
            ):
                w1t_hi = pre.tile([128, HC * H], BF, tag="w1t_hi")
                nc.sync.dma_start(w1t_hi[:], w1t_hi_in[:, :])
                w1t_lo = pre.tile([128, HC * H], BF, tag="w1t_lo")
                nc.sync.dma_start(w1t_lo[:], w1t_lo_in[:, :])
                for f in range(8):
                    ech, ecl = [], []
                    for hc in range(HC):
                        eh = ecp.tile([128, 512], BF, tag=f"ech{hc}", name=f"ech{hc}")
                        nc.sync.dma_start(
                            eh[:], encT_hi_in[:, hc * BS * N + f * 512:hc * BS * N + (f + 1) * 512])
                        ech.append(eh)
                        el = ecp.tile([128, 512], BF, tag=f"ecl{hc}", name=f"ecl{hc}")
                        nc.sync.dma_start(
                            el[:], encT_lo_in[:, hc * BS * N + f * 512:hc * BS * N + (f + 1) * 512])
                        ecl.append(el)
                    for kc in range(HC):
                        pp = psp.tile([128, 512], FP, tag="pp")
                        nmm = 3 * HC
                        i = 0
                        for hc in range(HC):
                            ks = slice(hc * H + kc * 128, hc * H + (kc + 1) * 128)
                            for wt, e in ((w1t_hi, ech[hc]), (w1t_hi, ecl[hc]),
                                          (w1t_lo, ech[hc])):
                                nc.tensor.matmul(
                                    pp[:], wt[:, ks], e[:],
                                    start=(i == 0), stop=(i == nmm - 1))
                                i += 1
                        eng = nc.vector if (kc % 2 == 0) else nc.scalar
                        if eng is nc.vector:
                            eng.tensor_copy(w1e[kc][:, f * 512:(f + 1) * 512], pp[:])
                        else:
                            eng.copy(w1e[kc][:, f * 512:(f + 1) * 512], pp[:])

            # ---- step-loop pools (opened after precompute pools closed) ----
            import contextlib
            _stack = contextlib.ExitStack()
            work = _stack.enter_context(tc.tile_pool(name="work", bufs=2))
            bigS = _stack.enter_context(tc.tile_pool(name="bigS", bufs=2))
            bigT = _stack.enter_context(tc.tile_pool(name="bigT", bufs=2))
            ps_g = _stack.enter_context(tc.tile_pool(name="ps_g", bufs=1, space="PSUM"))
            ps_u = _stack.enter_context(tc.tile_pool(name="ps_u", bufs=1, space="PSUM"))
            ps_s = _stack.enter_context(tc.tile_pool(name="ps_s", bufs=1, space="PSUM"))

            # ---- persistent state ----
            hT = state.tile([128, HC * BS], FP, tag="hT")      # [hp, (c,b)]
            xT = state.tile([128, HC * BS], FP, tag="xT")
            h_nat = state.tile([BS, H], FP, tag="h_nat")       # [b, h]
            maskneg = state.tile([BS, N], FP, tag="maskneg")
            scol = state.tile([BS, T], FP, tag="scol")
            eucol = state.tile([BS, T], FP, tag="eucol")
            mxcol = state.tile([BS, T], FP, tag="mxcol")
            idxcol = state.tile([BS, T], I32, tag="idxcol")
            nc.vector.memset(hT[:], 0.0)
            nc.vector.memset(xT[:], 0.0)
            nc.vector.memset(h_nat[:], 0.0)
            nc.vector.memset(maskneg[:], 0.0)

            def split_bf16(src_fp, pfx, t):
                hi = state.tile([128, HC * BS], BF, tag=f"{pfx}h{(t % 2) + 1}",
                                name=f"{pfx}hi{t % 2}")
                nc.vector.tensor_copy(hi[:], src_fp[:])
                lo = state.tile([128, HC * BS], BF, tag=f"{pfx}l{(t % 2) + 1}",
                                name=f"{pfx}lo{t % 2}")
                nc.vector.tensor_tensor(lo[:], src_fp[:], hi[:], op=OP.subtract)
                return hi, lo

            xb_hi = state.tile([128, HC * BS], BF, tag="xbh0")
            xb_lo = state.tile([128, HC * BS], BF, tag="xbl0")
            hb_hi = state.tile([128, HC * BS], BF, tag="hbh0")
            hb_lo = state.tile([128, HC * BS], BF, tag="hbl0")
            nc.vector.memset(xb_hi[:], 0.0)
            nc.vector.memset(xb_lo[:], 0.0)
            nc.vector.memset(hb_hi[:], 0.0)
            nc.vector.memset(hb_lo[:], 0.0)

            # natural-layout gate matmuls: out[b, j]; stationary = xT/hT chunk
            # (16 bf16 cols -> cheap LDW), moving = bf16 gate weight rows.
            # fp32 accuracy via 3-pass hi/lo compensation (hi*hi + hi*lo + lo*hi).
            def g_passes(s_hi, s_lo):
                return ((s_hi, gw_hi), (s_hi, gw_lo), (s_lo, gw_hi))

            def emit_gates_h_jb(rz_ps, jb, s_hi, s_lo):
                i = 0
                for c in range(HC):
                    for st, wt in g_passes(s_hi, s_lo):
                        nc.tensor.matmul(
                            rz_ps[:, jb * 512:(jb + 1) * 512],
                            st[:, c * BS:(c + 1) * BS],
                            wt[:, (4 + c) * 3 * H + jb * 512:(4 + c) * 3 * H + (jb + 1) * 512],
                            start=(i == 0), stop=False)
                        i += 1
                for br in (brow_hi, brow_lo):
                    nc.tensor.matmul(
                        rz_ps[:, jb * 512:(jb + 1) * 512], ones16[:],
                        br[:, jb * 512:(jb + 1) * 512],
                        start=False, stop=False)

            def emit_gates_h_n(hn_ps, in_ps, s_hi, s_lo):
                for k, br in enumerate((brow_hi, brow_lo)):
                    nc.tensor.matmul(in_ps[:, :], ones16[:],
                                     br[:, 2 * 512:3 * 512],
                                     start=(k == 0), stop=False)
                i = 0
                for c in range(HC):
                    for st, wt in g_passes(s_hi, s_lo):
                        nc.tensor.matmul(
                            hn_ps[:, :],
                            st[:, c * BS:(c + 1) * BS],
                            wt[:, (4 + c) * 3 * H + 2 * 512:(4 + c) * 3 * H + 3 * 512],
                            start=(i == 0), stop=False)
                        i += 1
                for k, br in enumerate((bhn_row_hi, bhn_row_lo)):
                    nc.tensor.matmul(hn_ps[:, :], ones16[:], br[:, :],
                                     start=False, stop=(k == 1))

            def emit_gates_h(rz_ps, hn_ps, in_ps, s_hi, s_lo):
                emit_gates_h_jb(rz_ps, 0, s_hi, s_lo)
                emit_gates_h_jb(rz_ps, 1, s_hi, s_lo)
                emit_gates_h_n(hn_ps, in_ps, s_hi, s_lo)

            def emit_gates_x(rz_ps, in_ps, s_hi, s_lo):
                for jb in range(2):
                    i = 0
                    for c in range(HC):
                        for st, wt in g_passes(s_hi, s_lo):
                            nc.tensor.matmul(
                                rz_ps[:, jb * 512:(jb + 1) * 512],
                                st[:, c * BS:(c + 1) * BS],
                                wt[:, c * 3 * H + jb * 512:c * 3 * H + (jb + 1) * 512],
                                start=False, stop=(i == 3 * HC - 1))
                            i += 1

                i = 0
                for c in range(HC):
                    for st, wt in g_passes(s_hi, s_lo):
                        nc.tensor.matmul(
                            in_ps[:, :],
                            st[:, c * BS:(c + 1) * BS],
                            wt[:, c * 3 * H + 2 * 512:c * 3 * H + 3 * 512],
                            start=False, stop=(i == 3 * HC - 1))
                        i += 1

            rz_ps = ps_g.tile([BS, 2 * H], FP, tag="rz")
            hn_ps = ps_g.tile([BS, H], FP, tag="hn")
            in_ps = ps_g.tile([BS, H], FP, tag="inn")
            emit_gates_h(rz_ps, hn_ps, in_ps, hb_hi, hb_lo)

            for t in range(T):
                emit_gates_x(rz_ps, in_ps, xb_hi, xb_lo)

                # ---- gate nonlinearities, natural layout, chunked by c so
                # the h' transposes can start before the whole chain is done
                trz = work.tile([BS, 2 * H], FP, tag="trz", bufs=1)
                rz = work.tile([BS, 2 * H], FP, tag="rz_sb", bufs=1)
                rhn = work.tile([BS, H], FP, tag="rhn", bufs=1)
                npre = work.tile([BS, H], FP, tag="npre", bufs=1)
                nn_t = work.tile([BS, H], FP, tag="nn", bufs=1)
                hmn = work.tile([BS, H], FP, tag="hmn", bufs=1)
                h_new = state.tile([BS, H], FP, tag=f"hn{(t % 2) + 1}",
                                   name=f"hnat{t % 2}")
                tr_ps = ps_s.tile([128, 2 * HC * BS], FP, tag="htp")
                ht_ps = tr_ps[:, 0:HC * BS]
                for c in range(HC):
                    sl = slice(c * 128, (c + 1) * 128)
                    slz = slice(H + c * 128, H + (c + 1) * 128)
                    nc.scalar.activation(trz[:, sl], rz_ps[:, sl], AF.Tanh, scale=0.5)
                    nc.scalar.activation(trz[:, slz], rz_ps[:, slz], AF.Tanh, scale=0.5)
                    nc.vector.tensor_scalar(rz[:, sl], trz[:, sl], 0.5, 0.5,
                                            OP.mult, OP.add)
                    nc.vector.tensor_scalar(rz[:, slz], trz[:, slz], 0.5, 0.5,
                                            OP.mult, OP.add)
                    nc.vector.tensor_tensor(rhn[:, sl], rz[:, sl], hn_ps[:, sl],
                                            op=OP.mult)
                    nc.vector.tensor_tensor(npre[:, sl], rhn[:, sl], in_ps[:, sl],
                                            op=OP.add)
                    nc.scalar.activation(nn_t[:, sl], npre[:, sl], AF.Tanh)
                    nc.vector.tensor_tensor(hmn[:, sl], h_nat[:, sl], nn_t[:, sl],
                                            op=OP.subtract)
                    nc.vector.tensor_tensor(hmn[:, sl], rz[:, slz], hmn[:, sl],
                                            op=OP.mult)
                    nc.vector.tensor_tensor(h_new[:, sl], nn_t[:, sl], hmn[:, sl],
                                            op=OP.add)
                    nc.tensor.matmul(
                        ht_ps[:, c * BS:(c + 1) * BS],
                        h_new[:, sl],
                        ident16[:], is_transpose=True,
                        start=(c == 0), stop=(c == HC - 1))
                h_nat = h_new
                hT_new = state.tile([128, HC * BS], FP, tag=f"hT{(t % 2) + 1}",
                                    name=f"hT{t % 2}")
                nc.vector.tensor_copy(hT_new[:], ht_ps[:])
                hT = hT_new
                hb_hi, hb_lo = split_bf16(hT, "hb", t)

                # ---- W2h: natural matmul then transpose to [h, b] ----
                w2n_ps = ps_s.tile([BS, H], FP, tag="w2")
                i = 0
                for c in range(HC):
                    for st, wt in ((hb_hi, w2t_hi), (hb_hi, w2t_lo), (hb_lo, w2t_hi)):
                        nc.tensor.matmul(
                            w2n_ps[:, :],
                            st[:, c * BS:(c + 1) * BS],
                            wt[:, c * H:(c + 1) * H],
                            start=(i == 0), stop=(i == 3 * HC - 1))
                        i += 1
                w2n = work.tile([BS, H], FP, tag="w2n", bufs=1)
                nc.vector.tensor_copy(w2n[:], w2n_ps[:])
                w2_ps = tr_ps[:, HC * BS:2 * HC * BS]
                for c in range(HC):
                    nc.tensor.matmul(
                        w2_ps[:, c * BS:(c + 1) * BS],
                        w2n[:, c * 128:(c + 1) * 128],
                        ident16[:], is_transpose=True,
                        start=(c == 0), stop=(c == HC - 1))
                w2h = work.tile([128, HC * BS], FP, tag="w2h")
                nc.vector.tensor_copy(w2h[:], w2_ps[:])

                # first half of next step's h-gates: fills the PE during the
                # preadd/tanh warm-up of the attention below
                if t < T - 1:
                    rz_ps = ps_g.tile([BS, 2 * H], FP, tag="rz")
                    hn_ps = ps_g.tile([BS, H], FP, tag="hn")
                    in_ps = ps_g.tile([BS, H], FP, tag="inn")
                    emit_gates_h_jb(rz_ps, 0, hb_hi, hb_lo)

                # ---- attention ----
                u_ps = ps_u.tile([BS, N], FP, tag="u")
                first = True
                for hc in range(HC):
                    for half in range(2):
                        S = bigS.tile([128, 8 * N], FP, tag="S")
                        s3 = S[:].rearrange("p (b n) -> p b n", n=N)
                        w1s = w1e[hc][:, half * 8 * N:(half + 1) * 8 * N].rearrange(
                            "p (b n) -> p b n", n=N)
                        wb = w2h[:, hc * BS + half * 8: hc * BS + half * 8 + 8]
                        pstep = wb.ap[0][0]
                        cstep = wb.ap[1][0]
                        nc.vector.tensor_tensor(
                            s3[:, 0:DVE_B, :], w1s[:, 0:DVE_B, :],
                            bass.AP(wb.tensor, wb.offset,
                                    [[pstep, 128], [cstep, DVE_B], [0, N]]),
                            op=OP.add)
                        nc.gpsimd.tensor_tensor(
                            s3[:, DVE_B:8, :], w1s[:, DVE_B:8, :],
                            bass.AP(wb.tensor, wb.offset + DVE_B * cstep,
                                    [[pstep, 128], [cstep, 8 - DVE_B], [0, N]]),
                            op=OP.add)
                        Tt = bigT.tile([128, 8 * N], FP, tag="Tt")
                        nc.scalar.activation(Tt[:], S[:], AF.Tanh)
                        for bl in range(8):
                            b = half * 8 + bl
                            last = (hc == HC - 1) and (half == 1) and (bl == 7)
                            nc.tensor.matmul(
                                u_ps[:, :],
                                ve[:, (hc * BS + b) * BS:(hc * BS + b + 1) * BS],
                                Tt[:, bl * N:(bl + 1) * N],
                                start=first, stop=last)
                            first = False

                # ---- argmax + gather issue (critical path first) ----
                up = work.tile([BS, N], FP, tag="up", bufs=1)
                nc.vector.tensor_tensor(up[:], u_ps[:], maskneg[:], op=OP.add)
                m8 = work.tile([BS, 8], FP, tag="m8")
                nc.vector.max(out=m8[:], in_=up[:])
                i8 = work.tile([BS, 8], U32, tag="i8")
                nc.vector.max_index(out=i8[:], in_max=m8[:], in_values=up[:])
                idxf = work.tile([BS, 1], FP, tag="idxf")
                nc.vector.tensor_copy(idxf[:], i8[:, 0:1])
                if t < T - 1:
                    offsf = work.tile([BS, 1], FP, tag="offsf")
                    nc.vector.tensor_scalar(offsf[:], idxf[:], bvec[:, 0:1], None, OP.add)
                    offs = work.tile([BS, 1], I32, tag="offs")
                    nc.vector.tensor_copy(offs[:], offsf[:])
                    prev = work.tile([BS, H], FP, tag="prev", bufs=1)
                    nc.gpsimd.indirect_dma_start(
                        out=prev[:], out_offset=None,
                        in_=enc_flat[:, :],
                        in_offset=bass.IndirectOffsetOnAxis(ap=offs[:, :1], axis=0))

                # second half of next step's h-gates: fills the PE while
                # softmax stats + gather DMA are in flight
                if t < T - 1:
                    emit_gates_h_jb(rz_ps, 1, hb_hi, hb_lo)
                    emit_gates_h_n(hn_ps, in_ps, hb_hi, hb_lo)

                # ---- softmax stats / mask update ----
                negmx = work.tile([BS, 1], FP, tag="negmx")
                nc.vector.tensor_scalar_mul(negmx[:], m8[:, 0:1], -1.0)
                nc.vector.tensor_copy(mxcol[:, t:t + 1], m8[:, 0:1])
                nc.vector.tensor_copy(idxcol[:, t:t + 1], i8[:, 0:1])
                ee = work.tile([BS, N], FP, tag="ee", bufs=1)
                nc.scalar.activation(ee[:], up[:], AF.Exp, bias=negmx[:, 0:1],
                                     accum_out=scol[:, t:t + 1])
                escr = work.tile([BS, N], FP, tag="escr", bufs=1)
                nc.vector.tensor_tensor(escr[:], ee[:], up[:], op=OP.mult)
                nc.vector.reduce_sum(out=eucol[:, t:t + 1], in_=escr[:], axis=AX.X)
                ohn = work.tile([BS, N], FP, tag="ohn", bufs=1)
                nc.vector.tensor_tensor(ohn[:], iota[:],
                                        idxf[:].to_broadcast([BS, N]),
                                        op=OP.is_equal)
                nc.vector.tensor_scalar_mul(ohn[:], ohn[:], NEG)
                nc.vector.tensor_tensor(maskneg[:], maskneg[:], ohn[:], op=OP.add)

                # ---- transpose gathered prev_embed ----
                if t < T - 1:
                    xt_ps = ps_s.tile([128, HC * BS], FP, tag="xtp")
                    for c in range(HC):
                        nc.tensor.matmul(
                            xt_ps[:, c * BS:(c + 1) * BS],
                            prev[:, c * 128:(c + 1) * 128],
                            ident16[:], is_transpose=True,
                            start=(c == 0), stop=(c == HC - 1))
                    xT_new = state.tile([128, HC * BS], FP, tag=f"xT{(t % 2) + 1}",
                                        name=f"xT{t % 2}")
                    nc.vector.tensor_copy(xT_new[:], xt_ps[:])
                    xT = xT_new
                    xb_hi, xb_lo = split_bf16(xT, "xb", t)

            # ---- epilogue: logs, entropy, outputs ----
            logs = state.tile([BS, T], FP, tag="logs")
            nc.scalar.activation(logs[:], scol[:], AF.Ln)
            rs = state.tile([BS, T], FP, tag="rs")
            nc.vector.reciprocal(rs[:], scol[:])
            t1 = state.tile([BS, T], FP, tag="t1")
            nc.vector.tensor_tensor(t1[:], mxcol[:], scol[:], op=OP.mult)
            nc.vector.tensor_tensor(t1[:], eucol[:], t1[:], op=OP.subtract)
            nc.vector.tensor_tensor(t1[:], t1[:], rs[:], op=OP.mult)
            ent_sb = state.tile([BS, T], FP, tag="ent")
            nc.vector.tensor_tensor(ent_sb[:], logs[:], t1[:], op=OP.subtract)
            logp_sb = state.tile([BS, T], FP, tag="logp")
            nc.vector.tensor_scalar_mul(logp_sb[:], logs[:], -1.0)
            nc.sync.dma_start(idx_out[:, :], idxcol[:])
            nc.sync.dma_start(logp_out[:, :], logp_sb[:])
            nc.sync.dma_start(ent_out[:, :], ent_sb[:])
            _stack.close()

    return nc


def _split16(a):
    import ml_dtypes
    hi = a.astype(ml_dtypes.bfloat16)
    lo = (a.astype(np.float32) - hi.astype(np.float32)).astype(ml_dtypes.bfloat16)
    return hi, lo


def _host_prep(enc_shard, w_ih, w_hh, b_ih, b_hh, W1, W2, v):
    """Build the per-core input map (enc_shard: (BS, N, H))."""
    f32 = lambda x: np.ascontiguousarray(x, dtype=np.float32)
    encT = f32(enc_shard.transpose(2, 0, 1).reshape(HC, 128, BS * N)
               .transpose(1, 0, 2).reshape(128, HC * BS * N))
    encT_hi, encT_lo = _split16(encT)
    enc_flat = f32(enc_shard.reshape(BS * N, H))
    w1t = f32(W1.T.reshape(HC, 128, H).transpose(1, 0, 2).reshape(128, HC * H))
    w1t_hi, w1t_lo = _split16(w1t)
    gws = np.concatenate([w_ih.T, w_hh.T], axis=0)  # (1024, 1536)
    gw = f32(gws.reshape(8, 128, 3 * H).transpose(1, 0, 2).reshape(128, 8 * 3 * H))
    gw_hi, gw_lo = _split16(gw)
    w2t = f32(W2.T.reshape(HC, 128, H).transpose(1, 0, 2).reshape(128, HC * H))
    w2t_hi, w2t_lo = _split16(w2t)
    ve = np.zeros((128, HC, BS, BS), np.float32)
    for hc in range(HC):
        for b in range(BS):
            ve[:, hc, b, b] = v[hc * 128:(hc + 1) * 128]
    ve = f32(ve.reshape(128, HC * BS * BS))
    # bias rows: rz part uses b_ih+b_hh; n part (cols 1024:1536) uses b_ih only
    brow = f32(np.concatenate([(b_ih + b_hh)[:2 * H], b_ih[2 * H:]])[None, :])
    brow_hi, brow_lo = _split16(brow)
    bhnrow = f32(b_hh[2 * H:][None, :])
    bhnrow_hi, bhnrow_lo = _split16(bhnrow)
    import ml_dtypes
    ones16 = np.ones((1, BS), ml_dtypes.bfloat16)
    iota = f32(np.broadcast_to(np.arange(N, dtype=np.float32)[None, :], (BS, N)))
    bvec = f32((np.arange(BS, dtype=np.float32) * N)[:, None])
    ident16 = np.eye(BS, dtype=np.float32)
    return {
        "encT_hi": encT_hi, "encT_lo": encT_lo, "encflat": enc_flat,
        "w1t_hi": w1t_hi, "w1t_lo": w1t_lo,
        "gw_hi": gw_hi, "gw_lo": gw_lo, "w2t_hi": w2t_hi, "w2t_lo": w2t_lo,
        "ve": ve, "ones16": ones16, "brow_hi": brow_hi, "brow_lo": brow_lo,
        "bhnrow_hi": bhnrow_hi, "bhnrow_lo": bhnrow_lo,
        "iota": iota, "bvec": bvec, "ident16": ident16,
    }


_NC_CACHE = {}


def kernel(encoder_outputs, w_ih, w_hh, b_ih, b_hh, W1, W2, v, decode_steps):
    T = int(decode_steps)
    enc = np.asarray(encoder_outputs, dtype=np.float32)
    args = [np.asarray(a, dtype=np.float32)
            for a in (w_ih, w_hh, b_ih, b_hh, W1, W2, v)]
    assert enc.shape == (B, N, H)

    if T not in _NC_CACHE:
        nc = build(T)
        nc.finalize()
        _NC_CACHE[T] = nc
    nc = _NC_CACHE[T]

    in_maps = []
    for c in range(N_CORES):
        shard = enc[c * BS:(c + 1) * BS]
        in_maps.append(_host_prep(shard, *args))

    res = run_bass_kernel_spmd(nc, in_maps, core_ids=list(range(N_CORES)))
    outs = res.results
    idx = np.concatenate([o["idx_out"] for o in outs], axis=0).astype(np.int32)
    logp = np.concatenate([o["logp_out"] for o in outs], axis=0).astype(np.float32)
    ent = np.concatenate([o["ent_out"] for o in outs], axis=0).astype(np.float32)
    return idx, logp, ent
